# revision 5
# baseline (speedup 1.0000x reference)
"""Trainium2 Bass kernel for the dense RandLA-Net block — dma_gather design v8 (v7 + deeper tail gather splitting).

v4 vs v3: per-tile geometry runs fully before the chunk loop so the Scalar
engine does [Sqrt x4, Exp x16] per tile (2 activation-table loads instead of
~6); all relus moved to the vector engine; ft triple-buffered so the
per-tile dma_gather (the pacer, ~64us of GpSimd descriptor generation)
overlaps compute of the previous tiles.
"""

import sys

import numpy as np

sys.path.insert(0, "/opt/trn_rl_repo")

import ml_dtypes

import concourse.bass as bass
import concourse.tile as tile
from concourse import mybir, bacc
from concourse.bass_utils import run_bass_kernel_spmd

F32 = mybir.dt.float32
BF16 = mybir.dt.bfloat16
I16 = mybir.dt.int16
AF = mybir.ActivationFunctionType
OP = mybir.AluOpType
BF = ml_dtypes.bfloat16

B, C_IN, N, K = 4, 64, 16384, 16
D_REL, C_MID, C_OUT = 64, 128, 128
NP = N // 2            # points per core
PK = NP * K            # columns per core (131072)
NT = 16                # tiles (point blocks of 512)
LT = PK // NT          # 8192 cols per tile
NCH = 16               # k-chunks per tile
LC = 512               # chunk cols


def _build_kernel():
    nc = bacc.Bacc("TRN2", target_bir_lowering=False,
                   dynamic_dma_scratch_size=32768)

    tab = nc.dram_tensor("tab", [N, 128], BF16, kind="ExternalInput")
    gidx = nc.dram_tensor("gidx", [128, NP], I16, kind="ExternalInput")
    posCB4 = nc.dram_tensor("posCB4", [3, 4 * NP], BF16, kind="ExternalInput")
    wgeo = nc.dram_tensor("wgeo", [3, 160], BF16, kind="ExternalInput")
    wd128 = nc.dram_tensor("wd128", [128, 256], BF16, kind="ExternalInput")
    watt = nc.dram_tensor("watt", [128, 128], BF16, kind="ExternalInput")
    wglob = nc.dram_tensor("wglob", [128, 128], BF16, kind="ExternalInput")
    ident = nc.dram_tensor("ident", [128, 128], BF16, kind="ExternalInput")
    brel = nc.dram_tensor("brel", [64, 1], F32, kind="ExternalInput")
    bglob = nc.dram_tensor("bglob", [128, 1], F32, kind="ExternalInput")
    outp = nc.dram_tensor("outp", [128, NP], F32, kind="ExternalOutput")

    with tile.TileContext(nc) as tc:
        with tc.tile_pool(name="persist", bufs=1) as pp:
            gidx_sb = pp.tile([128, NP], I16)
            posCB4_sb = pp.tile([3, 4 * NP], BF16)
            wgeo_sb = pp.tile([3, 160], BF16)
            wd128_sb = pp.tile([128, 256], BF16)
            watt_sb = pp.tile([128, 128], BF16)
            wglob_sb = pp.tile([128, 128], BF16)
            ident_sb = pp.tile([128, 128], BF16)
            brel_sb = pp.tile([64, 1], F32)
            bglob_sb = pp.tile([128, 1], F32)
            nc.sync.dma_start(out=gidx_sb[:, 0:512], in_=gidx.ap()[:, 0:512])
            nc.sync.dma_start(out=gidx_sb[:, 512:NP], in_=gidx.ap()[:, 512:NP])
            nc.sync.dma_start(out=posCB4_sb, in_=posCB4.ap())
            nc.sync.dma_start(out=wgeo_sb, in_=wgeo.ap())
            nc.sync.dma_start(out=wd128_sb, in_=wd128.ap())
            nc.sync.dma_start(out=watt_sb, in_=watt.ap())
            nc.sync.dma_start(out=wglob_sb, in_=wglob.ap())
            nc.sync.dma_start(out=ident_sb, in_=ident.ap())
            nc.sync.dma_start(out=brel_sb, in_=brel.ap())
            nc.sync.dma_start(out=bglob_sb, in_=bglob.ap())

            with tc.tile_pool(name="ftiles", bufs=3) as fp, \
                 tc.tile_pool(name="geo", bufs=2) as gp, \
                 tc.tile_pool(name="dsbs", bufs=9) as dp, \
                 tc.tile_pool(name="chunks", bufs=2) as cp, \
                 tc.tile_pool(name="mps", bufs=2, space="PSUM") as mpsum, \
                 tc.tile_pool(name="dnps", bufs=1, space="PSUM") as dpsum:
                def emit_gather(t):
                    ft = fp.tile([128, LT], BF16, tag="ft")
                    nhalf = {NT - 2: 2, NT - 1: 4}.get(t, 1)
                    nidx = LT // nhalf
                    for h in range(nhalf):
                        out_h = ft[:, h * nidx:(h + 1) * nidx]
                        nc.gpsimd.dma_gather(
                            out_ap=out_h.rearrange("p (a i) -> p a i", a=1),
                            in_ap=tab.ap(),
                            idxs_ap=gidx_sb[:, t * 512 + h * (nidx // 16):
                                            t * 512 + (h + 1) * (nidx // 16)],
                            num_idxs=nidx,
                            num_idxs_reg=nidx,
                            elem_size=128,
                            transpose=True,
                            single_packet=False,
                        )
                    return ft

                def emit_geo(t, ft):
                    cen = posCB4_sb[:, t * 2048:(t + 1) * 2048]
                    dsbs = []
                    for g4 in range(4):
                        gcols = slice(g4 * 2048, (g4 + 1) * 2048)
                        rel = gp.tile([3, 2048], BF16, tag="rel")
                        nc.vector.tensor_tensor(
                            out=rel, in0=ft[0:3, gcols], in1=cen,
                            op=OP.subtract)
                        nc.vector.tensor_mul(rel, rel, rel)
                        psd = mpsum.tile([128, 512], F32, tag="psd")
                        for o in range(4):
                            nc.tensor.matmul(
                                psd[32 * o:32 * o + 32, :],
                                wgeo_sb[:, 128:160],
                                rel[:, 512 * o:512 * (o + 1)],
                                start=True, stop=True,
                                tile_position=(0, 32 * o),
                                skip_group_check=True)
                        dsb = dp.tile([128, 512], BF16, tag="dsb")
                        nc.scalar.activation(out=dsb, in_=psd, func=AF.Sqrt)
                        dsbs.append(dsb)
                    return dsbs

                def emit_chunks(t, ft, dsbs):
                    ps_dn = dpsum.tile([128, 1024], F32, tag="dn")
                    for c in range(NCH):
                        g4, o = c // 4, c % 4
                        cols = slice(c * 512, (c + 1) * 512)
                        ps_rp = mpsum.tile([128, 512], F32, tag="rp")
                        nc.tensor.matmul(
                            ps_rp[0:64, :], wgeo_sb[:, 0:64],
                            ft[0:3, cols],
                            start=True, stop=False,
                            tile_position=(0, 0),
                            skip_group_check=True)
                        nc.tensor.matmul(
                            ps_rp[0:64, :], wgeo_sb[:, 64:128],
                            posCB4_sb[0:3, t * 2048:t * 2048 + 512],
                            start=False, stop=False,
                            tile_position=(0, 0),
                            skip_group_check=True)
                        nc.tensor.matmul(
                            ps_rp[0:64, :],
                            wd128_sb[:, 64 * o:64 * (o + 1)],
                            dsbs[g4][:, :],
                            start=False, stop=True,
                            tile_position=(0, 0),
                            skip_group_check=True)
                        nc.vector.tensor_scalar(
                            out=ft[0:64, cols], in0=ps_rp[0:64, :],
                            scalar1=brel_sb, scalar2=0.0,
                            op0=OP.add, op1=OP.max)
                        ps_s = mpsum.tile([128, 512], F32, tag="sc")
                        nc.tensor.matmul(ps_s, watt_sb, ft[:, cols],
                                         start=True, stop=True,
                                         skip_group_check=True)
                        eu = cp.tile([128, 1024], BF16, tag="eu")
                        nc.scalar.activation(out=eu[:, 0:512], in_=ps_s,
                                             func=AF.Exp)
                        nc.vector.tensor_mul(eu[:, 512:1024], ft[:, cols],
                                             eu[:, 0:512])
                        nc.tensor.matmul(ps_dn[:, 0:512], ident_sb,
                                         eu[:, 0:512],
                                         start=(c == 0), stop=(c == NCH - 1),
                                         skip_group_check=True)
                        nc.tensor.matmul(ps_dn[:, 512:1024], ident_sb,
                                         eu[:, 512:1024],
                                         start=(c == 0), stop=(c == NCH - 1),
                                         skip_group_check=True)
                    rcp = cp.tile([128, 512], F32, tag="rcp")
                    nc.vector.reciprocal(rcp, ps_dn[:, 0:512])
                    agg = cp.tile([128, 512], BF16, tag="agg")
                    nc.vector.tensor_mul(agg, ps_dn[:, 512:1024], rcp)
                    ps_o = mpsum.tile([128, 512], F32, tag="sc")
                    nc.tensor.matmul(ps_o, wglob_sb, agg, start=True, stop=True,
                                     skip_group_check=True)
                    osb = cp.tile([128, 512], F32, tag="osb")
                    nc.vector.tensor_scalar(
                        out=osb, in0=ps_o, scalar1=bglob_sb, scalar2=0.0,
                        op0=OP.add, op1=OP.max)
                    nc.sync.dma_start(out=outp.ap()[:, t * 512:(t + 1) * 512],
                                      in_=osb)

                GMS = 0.075  # modeled real gather cadence, ms
                fts = {0: emit_gather(0), 1: emit_gather(1)}
                with tc.tile_wait_until(GMS):
                    dsb_cur = emit_geo(0, fts[0])
                for t in range(NT):
                    if t + 2 < NT:
                        fts[t + 2] = emit_gather(t + 2)
                    emit_chunks(t, fts[t], dsb_cur)
                    if t + 1 < NT:
                        with tc.tile_wait_until((t + 2) * GMS):
                            dsb_cur = emit_geo(t + 1, fts[t + 1])
                    fts.pop(t)
    nc.compile()
    return nc


_NC = None


def _get_nc():
    global _NC
    if _NC is None:
        _NC = _build_kernel()
    return _NC


def _prep_batch(xb, posb, W_att, W_glob, W_rel, b_rel, b_glob):
    """Per-batch shared tensors (same for both halves)."""
    tab = np.zeros((N, 128), dtype=BF)
    tab[:, 0:3] = posb.astype(BF)
    tab[:, 64:128] = xb.T.astype(BF)

    Wc, Wn, Wr, wd = W_rel[0:3], W_rel[3:6], W_rel[6:9], W_rel[9:10]
    wgeo = np.zeros((3, 160), dtype=BF)
    wgeo[:, 0:64] = (Wn + Wr).astype(BF)
    wgeo[:, 64:128] = (Wc - Wr).astype(BF)
    wgeo[:, 128] = 1.0
    wd128 = np.zeros((128, 256), dtype=BF)
    for o in range(4):
        wd128[32 * o, 64 * o:64 * (o + 1)] = wd[0].astype(BF)
    return {
        "tab": tab, "wgeo": wgeo, "wd128": wd128,
        "watt": W_att.astype(BF), "wglob": W_glob.astype(BF),
        "ident": np.eye(128, dtype=BF),
        "brel": b_rel.reshape(64, 1).astype(np.float32),
        "bglob": b_glob.reshape(128, 1).astype(np.float32),
    }


def _prep_core(core, shared, pos, neigh):
    b = core // 2
    half = core % 2
    P0 = half * NP
    nb = neigh[b][P0:P0 + NP].astype(np.int64)      # [NP, K]

    # column j of tile t: k = j//512, i = j%512 -> idx = nb[512t+i, k]
    A = nb.reshape(NT, 512, K)                      # [t, i, k]
    V = A.transpose(0, 2, 1).reshape(NT, LT)        # [t, j]
    # wrapped in 16 partitions: j at partition j%16, col j//16
    W16 = V.reshape(NT, 512, 16).transpose(0, 2, 1)  # [t, p, col]
    gidx = np.tile(W16.transpose(1, 0, 2).reshape(16, NP), (8, 1)).astype(np.int16)

    pl = pos[b][P0:P0 + NP].astype(np.float32)      # [NP, 3]
    pc = pl.T                                        # [3, NP]
    posCB4 = np.repeat(
        pc.reshape(3, NT, 1, 512), 4, axis=2).reshape(3, 4 * NP).astype(BF)

    out = dict(shared[b])
    out["gidx"] = gidx
    out["posCB4"] = posCB4
    return out


def kernel(x, pos, neigh_idx, W_rel, b_rel, W_att, W_glob, b_glob, **kw):
    x = np.ascontiguousarray(np.asarray(x, dtype=np.float32))
    pos = np.ascontiguousarray(np.asarray(pos, dtype=np.float32))
    neigh = np.asarray(neigh_idx)
    W_rel = np.asarray(W_rel, dtype=np.float32)
    W_att = np.asarray(W_att, dtype=np.float32)
    W_glob = np.asarray(W_glob, dtype=np.float32)
    b_rel = np.asarray(b_rel, dtype=np.float32)
    b_glob = np.asarray(b_glob, dtype=np.float32)

    nc = _get_nc()
    shared = [
        _prep_batch(x[b], pos[b], W_att, W_glob, W_rel, b_rel, b_glob)
        for b in range(B)
    ]
    in_maps = [_prep_core(core, shared, pos, neigh) for core in range(8)]
    res = run_bass_kernel_spmd(nc, in_maps, core_ids=list(range(8)))
    out = np.zeros((B, C_OUT, N), np.float32)
    for core in range(8):
        b = core // 2
        P0 = (core % 2) * NP
        out[b, :, P0:P0 + NP] = res.results[core]["outp"]
    return out
